# revision 48
# baseline (speedup 1.0000x reference)
"""Trainium2 Bass kernel for a LoRA self-attention block (diffusers-style
CustomLoRAAttnProcessor).

  B=8, S=1024, D=1280, H=20 heads x HD=64, LoRA rank 4 (folded into the
  weights on the host: W_eff = W + 0.25 * B @ A, mathematically identical).

Distribution: pure data parallelism — one batch element per NeuronCore
(8 cores), no collectives.

Per-core layout choices (contraction dim always on SBUF partitions; host
pre-transposes x and the effective weights; all matmul operands bf16 —
fp32 matmuls cost 4x on TRN2):

  phase V : v token-major [S, D] with a ones-column per head (the AV
            matmul then also emits the softmax denominator for free).
  attention pipeline (per feature tile t = head pair 2t, 2t+1):
    - scoresT[k,q]: the two K=64 halves are issued back-to-back into
      distinct PSUM banks of one tile, so they run CONCURRENTLY on PE
      row groups 0-63/64-127 (2x) and the slot frees atomically with a
      single FD=1024 exp on ACT (scale=1/8 folded, no max-subtraction).
    - AV(t) is emitted one t late so all 32 matmuls are ready and run
      as dense PE runs (fewer weight-buffer transition stalls).
    - q/k projection mgroups are emitted as half-blocks tucked after the
      AV blocks so the ACT exp chain is never starved at boundaries.
    - normalize: sumexp row DMA-reshaped across 64 DVE lanes for the
      reciprocal, gpsimd partition_broadcast, bf16 throughout.
  phase O : split kk=0..7 partials (bias folded, bf16) that fill the
            ACT-bound t=8/9 stretch, then a short final pass (identity-
            matmul partial re-add + kk=8,9) so the post-attention tail
            is minimal. PE HAM warmup matmuls cover the initial DMA.
"""

import sys

for _p in ("/opt/trn_rl_repo",):
    if _p not in sys.path:
        sys.path.insert(0, _p)

from contextlib import ExitStack

import ml_dtypes
import numpy as np

import concourse.bass as bass  # noqa: F401  (import order: bass before tile)
import concourse.tile as tile
from concourse import bacc, mybir
from concourse.bass_utils import run_bass_kernel_spmd

B, S, D = 8, 1024, 1280
H, HD = 20, 64
SCALING = 0.25  # alpha / rank
ATTN_SCALE = 1.0 / 8.0  # 1/sqrt(HD)

DT = D // 128  # 10 feature tiles
KC = S // 128  # 8 key-position chunks
MG = 5  # weight column groups of 256 (2 output tiles each)
VW = HD + 1  # v columns per head incl. ones column

F32 = mybir.dt.float32
BF16 = mybir.dt.bfloat16
EXP = mybir.ActivationFunctionType.Exp

N_CORES = 8


def _w_stripe(nc, wpool, wdram, mg, name):
    """DMA a 256-wide column group of a weight matrix into SBUF,
    feature-major: stripe[p, t, n] = W[t*128+p, mg*256+n]."""
    stripe = wpool.tile([128, DT, 256], BF16, tag="w", name=name)
    nc.sync.dma_start(
        out=stripe,
        in_=wdram[:, mg * 256 : (mg + 1) * 256].rearrange("(t p) n -> p t n", p=128),
    )
    return stripe


def _qk_mgroup(nc, xT_sb, stripe, pp, dst, mg):
    """One 256-wide column group of a feature-major projection:
    dst[:, m, :] = (W.T @ x.T) for m in the group."""
    for ml in range(2):
        m = mg * 2 + ml
        ps0 = pp.tile([128, 512], F32, tag="pp", name=f"ps0_{m}")
        ps1 = pp.tile([128, 512], F32, tag="pp", name=f"ps1_{m}")
        for kk in range(DT):
            lhsT = stripe[:, kk, ml * 128 : (ml + 1) * 128]
            nc.tensor.matmul(
                ps0, lhsT=lhsT, rhs=xT_sb[:, kk, 0:512],
                start=(kk == 0), stop=(kk == DT - 1),
            )
            nc.tensor.matmul(
                ps1, lhsT=lhsT, rhs=xT_sb[:, kk, 512:1024],
                start=(kk == 0), stop=(kk == DT - 1),
            )
        nc.vector.tensor_copy(out=dst[:, m, 0:512], in_=ps0)
        nc.vector.tensor_copy(out=dst[:, m, 512:1024], in_=ps1)


def _emit(nc, tc, xT, wqT, wkT, wvT, woT, bo, ident, outT):
    persist_cm = tc.tile_pool(name="persist", bufs=1)
    persist = persist_cm.__enter__()
    qT_sb = persist.tile([128, DT, S], BF16)
    kT_sb = persist.tile([128, DT, S], BF16)
    v_sb = persist.tile([128, KC, H * VW], BF16)
    ctxT_sb = persist.tile([128, DT, S], BF16)
    bo_sb = persist.tile([128, DT, 1], F32)
    partial_sb = persist.tile([128, DT, 2, 512], BF16)
    ident_sb = persist.tile([128, 128], BF16)
    nc.sync.dma_start(out=bo_sb, in_=bo[:].rearrange("(t p) -> p t", p=128))
    nc.sync.dma_start(out=ident_sb, in_=ident[:, :])
    nc.vector.memset(
        v_sb[:].rearrange("p a (h c) -> p a h c", c=VW)[:, :, :, HD : HD + 1], 1.0
    )

    xpool_cm = tc.tile_pool(name="xpool", bufs=1)
    xpool = xpool_cm.__enter__()
    xT_sb = xpool.tile([128, DT, S], BF16)

    # wpool opened BEFORE phase V so the first q/k weight stripes do not
    # address-reuse (WAR) the wv region: their DMA can land during phase V.
    wpool_cm = tc.tile_pool(name="wpool", bufs=3)
    wpool = wpool_cm.__enter__()

    # ---------------- phase V: v projection (token-major) ----------------
    with ExitStack() as pv:
        wupool = pv.enter_context(tc.tile_pool(name="wupool", bufs=1))
        wups = pv.enter_context(tc.tile_pool(name="wups", bufs=1, space="PSUM"))
        vwpool = pv.enter_context(tc.tile_pool(name="vwpool", bufs=1))
        vp = pv.enter_context(tc.tile_pool(name="vp", bufs=4, space="PSUM"))
        vw = vwpool.tile([128, DT, D], BF16)

        # HAM warmup: dummy matmuls keep the PE busy during the initial
        # input DMA so real matmuls start at the 2.4 GHz clock.
        wu_sb = wupool.tile([128, 512], BF16)
        nc.vector.memset(wu_sb, 0.0)
        wu_ps = wups.tile([128, 512], F32)
        for _ in range(8):
            nc.tensor.matmul(
                wu_ps, lhsT=wu_sb[:, 0:128], rhs=wu_sb, start=True, stop=True
            )

        # interleave xT / wv tiles so the first V matmuls can start early
        for kk in range(DT):
            nc.sync.dma_start(
                out=xT_sb[:, kk, :], in_=xT[kk * 128 : (kk + 1) * 128, :]
            )
            nc.sync.dma_start(
                out=vw[:, kk, :], in_=wvT[kk * 128 : (kk + 1) * 128, :]
            )
        # prefetch the first q/k stripes behind the V-phase inputs
        sq = _w_stripe(nc, wpool, wqT, 0, "wq0")
        sk = _w_stripe(nc, wpool, wkT, 0, "wk0")
        NCH = ((0, 512), (512, 512), (1024, 256))
        for st in range(KC):
            for ni, (n0, nw) in enumerate(NCH):
                # chunk-serial: each output chunk is a dense run of 10
                # same-geometry matmuls (the N=256 chunk otherwise straggles
                # at ~2x its ideal issue rate when interleaved per kk).
                ps = vp.tile([128, nw], F32, tag="vp", name=f"vps{st}_{ni}")
                for kk in range(DT):
                    nc.tensor.matmul(
                        ps,
                        lhsT=xT_sb[:, kk, st * 128 : (st + 1) * 128],
                        rhs=vw[:, kk, n0 : n0 + nw],
                        start=(kk == 0), stop=(kk == DT - 1),
                    )
                if st == 0:
                    # DMA-gated stretch: keep the PE (and HAM) warm with
                    # dependency-free filler matmuls between tile arrivals.
                    for _ in range(12):
                        nc.tensor.matmul(
                            wu_ps, lhsT=wu_sb[:, 0:128], rhs=wu_sb,
                            start=True, stop=True,
                        )
                nh = nw // HD
                nc.vector.tensor_copy(
                    out=v_sb[
                        :, st, (n0 // HD) * VW : (n0 // HD + nh) * VW
                    ].rearrange("p (h c) -> p h c", c=VW)[:, :, 0:HD],
                    in_=ps[:].rearrange("p (h c) -> p h c", c=HD),
                )

    # -------- phase QK + attention, interleaved per weight column group ----
    with ExitStack() as p2:
        epool = p2.enter_context(tc.tile_pool(name="epool", bufs=3))
        small = p2.enter_context(tc.tile_pool(name="small", bufs=2))
        pp = p2.enter_context(tc.tile_pool(name="pp", bufs=2, space="PSUM"))
        sc = p2.enter_context(tc.tile_pool(name="sc", bufs=2, space="PSUM"))
        cx = p2.enter_context(tc.tile_pool(name="cx", bufs=2, space="PSUM"))

        def scores_exp(t):
            # exps laid out [128, kc, qc, half, 512] so one FD=1024 exp
            # covers the head pair of each (kc, qc) unit; two tiles per t
            # (kc 0-3 / 4-7) to keep the pool granularity small.
            ex_tiles = [
                epool.tile(
                    [128, KC // 2, 2, 2, 512], BF16, tag="exp", name=f"exp{t}_{i}"
                )
                for i in range(2)
            ]
            # scoresT + exp; head pair (2t, 2t+1) row-packs the PE.
            # The two K=64 halves are issued back-to-back into distinct
            # PSUM banks of ONE tile so they execute concurrently on row
            # groups 0-63 / 64-127 and their slot frees atomically.
            for kc in range(KC):
                for qc in range(2):
                    ps = sc.tile([128, 2, 512], F32, tag="sc", name="scps")
                    for half in range(2):
                        p0 = half * 64
                        nc.tensor.matmul(
                            ps[:, half, :],
                            lhsT=kT_sb[p0 : p0 + 64, t, kc * 128 : (kc + 1) * 128],
                            rhs=qT_sb[p0 : p0 + 64, t, qc * 512 : (qc + 1) * 512],
                            start=True,
                            stop=True,
                        )
                    nc.scalar.activation(
                        out=ex_tiles[kc // 4][:, kc % 4, qc, :, :],
                        in_=ps,
                        func=EXP,
                        scale=ATTN_SCALE,
                    )
            return ex_tiles

        def av_normalize(t, ex_tiles):
            # AV + sumexp + normalize; emitted one t later than its scores
            # so all 32 AV matmuls are ready and run as dense PE runs.
            for half in range(2):
                h = 2 * t + half
                stage = small.tile([HD + 1, S], BF16, tag="stage", name="stage")
                for qc in range(2):
                    cps = cx.tile([HD + 1, 512], F32, tag="cx", name="cxps")
                    for kc in range(KC):
                        nc.tensor.matmul(
                            cps,
                            lhsT=v_sb[:, kc, h * VW : (h + 1) * VW],
                            rhs=ex_tiles[kc // 4][:, kc % 4, qc, half, :],
                            start=(kc == 0),
                            stop=(kc == KC - 1),
                        )
                    nc.vector.tensor_copy(
                        out=stage[:, qc * 512 : (qc + 1) * 512], in_=cps
                    )
                # 1/sumexp: reshape the [1, S] row across 64 DVE lanes
                r64 = small.tile([64, 16], BF16, tag="r64", name="r64")
                nc.sync.dma_start(out=r64, in_=stage[HD : HD + 1, :])
                rc64 = small.tile([64, 16], BF16, tag="rc64", name="rc64")
                with nc.allow_low_precision(reason="bf16 softmax denom"):
                    nc.vector.reciprocal(rc64, r64)
                rrow = small.tile([1, S], BF16, tag="rrow", name="rrow")
                nc.sync.dma_start(out=rrow, in_=rc64)
                bcast = small.tile([HD, S], BF16, tag="bcast", name="bcast")
                nc.gpsimd.partition_broadcast(bcast, rrow)
                nc.vector.tensor_mul(
                    ctxT_sb[half * 64 : half * 64 + 64, t, :],
                    stage[0:HD, :],
                    bcast,
                )

        # mg0 projections run up front; later mgroups are emitted as q/k
        # half-blocks tucked after the AV blocks so the exp (ACT) chain is
        # never starved by a long projection stretch at mgroup boundaries.
        _qk_mgroup(nc, xT_sb, sq, pp, qT_sb, 0)
        _qk_mgroup(nc, xT_sb, sk, pp, kT_sb, 0)
        stripes = {
            (0, 1): _w_stripe(nc, wpool, wqT, 1, "wq1"),
            (1, 1): _w_stripe(nc, wpool, wkT, 1, "wk1"),
        }
        pending = None
        for t in range(DT):
            mgn = t // 2 + 1
            ex = scores_exp(t)
            if pending is not None:
                av_normalize(*pending)
            pending = (t, ex)
            if mgn < MG:
                which = t % 2  # 0 -> q of mgn, 1 -> k of mgn
                dst = qT_sb if which == 0 else kT_sb
                _qk_mgroup(nc, xT_sb, stripes.pop((which, mgn)), pp, dst, mgn)
                if mgn + 1 < MG:
                    wdram, nm = (wqT, "wq") if which == 0 else (wkT, "wk")
                    stripes[(which, mgn + 1)] = _w_stripe(
                        nc, wpool, wdram, mgn + 1, f"{nm}{mgn + 1}"
                    )
        av_normalize(*pending)

        # ---- O partials over kk=0..7 (heads 0..15): pure PE gap-filler for
        # the ACT-bound t=8/9 stretch. Bias is folded into the bf16 partial.
        for mg in range(MG):
            so = _w_stripe(nc, wpool, woT, mg, f"wo{mg}")
            for ml in range(2):
                m = mg * 2 + ml
                for qc in range(2):
                    ps = pp.tile([128, 512], F32, tag="pp", name="opart")
                    for kk in range(DT - 2):
                        nc.tensor.matmul(
                            ps,
                            lhsT=so[:, kk, ml * 128 : (ml + 1) * 128],
                            rhs=ctxT_sb[:, kk, qc * 512 : (qc + 1) * 512],
                            start=(kk == 0),
                            stop=(kk == DT - 3),
                        )
                    nc.vector.tensor_scalar_add(
                        partial_sb[:, m, qc, :], ps, bo_sb[:, m, :]
                    )

    wpool_cm.__exit__(None, None, None)
    xpool_cm.__exit__(None, None, None)

    # ------- phase O final: kk=8,9 + identity-matmul partial add -------
    with ExitStack() as p3:
        wpool3 = p3.enter_context(tc.tile_pool(name="wpool3", bufs=2))
        opp = p3.enter_context(tc.tile_pool(name="opp", bufs=4, space="PSUM"))
        ostage = p3.enter_context(tc.tile_pool(name="ostage", bufs=4))
        for mg in range(MG):
            stripe2 = wpool3.tile([128, 2, 256], BF16, tag="w3", name=f"w3_{mg}")
            nc.sync.dma_start(
                out=stripe2,
                in_=woT[(DT - 2) * 128 :, mg * 256 : (mg + 1) * 256].rearrange(
                    "(t p) n -> p t n", p=128
                ),
            )
            for ml in range(2):
                m = mg * 2 + ml
                for qc in range(2):
                    ps = opp.tile([128, 512], F32, tag="opp", name="opps")
                    nc.tensor.matmul(
                        ps,
                        lhsT=ident_sb,
                        rhs=partial_sb[:, m, qc, :],
                        start=True,
                        stop=False,
                    )
                    for j in range(2):
                        kk = DT - 2 + j
                        nc.tensor.matmul(
                            ps,
                            lhsT=stripe2[:, j, ml * 128 : (ml + 1) * 128],
                            rhs=ctxT_sb[:, kk, qc * 512 : (qc + 1) * 512],
                            start=False,
                            stop=(j == 1),
                        )
                    o_sb = ostage.tile([128, 512], BF16, tag="ostage", name="osb")
                    nc.vector.tensor_copy(o_sb, ps)
                    nc.sync.dma_start(
                        out=outT[m * 128 : (m + 1) * 128, qc * 512 : (qc + 1) * 512],
                        in_=o_sb,
                    )

    persist_cm.__exit__(None, None, None)


def build_nc():
    nc = bacc.Bacc(None, target_bir_lowering=False)
    xT = nc.dram_tensor("xT", [D, S], BF16, kind="ExternalInput")
    wqT = nc.dram_tensor("wqT", [D, D], BF16, kind="ExternalInput")
    wkT = nc.dram_tensor("wkT", [D, D], BF16, kind="ExternalInput")
    wvT = nc.dram_tensor("wvT", [D, D], BF16, kind="ExternalInput")
    woT = nc.dram_tensor("woT", [D, D], BF16, kind="ExternalInput")
    bo = nc.dram_tensor("bo", [D], F32, kind="ExternalInput")
    ident = nc.dram_tensor("ident", [128, 128], BF16, kind="ExternalInput")
    outT = nc.dram_tensor("outT", [D, S], BF16, kind="ExternalOutput")
    with tile.TileContext(nc) as tc:
        _emit(nc, tc, xT, wqT, wkT, wvT, woT, bo, ident, outT)
    nc.compile()
    return nc


_NC = None


def _get_nc():
    global _NC
    if _NC is None:
        _NC = build_nc()
    return _NC


def make_in_maps(hidden_states, Wq, Wk, Wv, Wo, bo, Aq, Bq, Ak, Bk, Av, Bv, Ao, Bo):
    x = np.asarray(hidden_states, dtype=np.float32)

    def eff_T(W, A, Bup):
        W64 = np.asarray(W, dtype=np.float64)
        lora = np.asarray(Bup, dtype=np.float64) @ np.asarray(A, dtype=np.float64)
        return np.ascontiguousarray(
            (W64 + SCALING * lora).T.astype(ml_dtypes.bfloat16)
        )

    base = {
        "wqT": eff_T(Wq, Aq, Bq),
        "wkT": eff_T(Wk, Ak, Bk),
        "wvT": eff_T(Wv, Av, Bv),
        "woT": eff_T(Wo, Ao, Bo),
        "bo": np.ascontiguousarray(np.asarray(bo, dtype=np.float32)),
        "ident": np.eye(128, dtype=ml_dtypes.bfloat16),
    }
    return [
        dict(base, xT=np.ascontiguousarray(x[b].T.astype(ml_dtypes.bfloat16)))
        for b in range(x.shape[0])
    ]


def kernel(**inputs):
    in_maps = make_in_maps(**inputs)
    nc = _get_nc()
    res = run_bass_kernel_spmd(nc, in_maps, core_ids=list(range(N_CORES)))
    out = np.stack([res.results[b]["outT"].T for b in range(N_CORES)])
    return np.ascontiguousarray(out, dtype=np.float32)



# revision 49
# speedup vs baseline: 1.0019x; 1.0019x over previous
"""Trainium2 Bass kernel for a LoRA self-attention block (diffusers-style
CustomLoRAAttnProcessor).

  B=8, S=1024, D=1280, H=20 heads x HD=64, LoRA rank 4 (folded into the
  weights on the host: W_eff = W + 0.25 * B @ A, mathematically identical).

Distribution: pure data parallelism — one batch element per NeuronCore
(8 cores), no collectives.

Per-core layout choices (contraction dim always on SBUF partitions; host
pre-transposes x and the effective weights; all matmul operands bf16 —
fp32 matmuls cost 4x on TRN2):

  phase V : v token-major [S, D] with a ones-column per head (the AV
            matmul then also emits the softmax denominator for free).
  attention pipeline (per feature tile t = head pair 2t, 2t+1):
    - scoresT[k,q]: the two K=64 halves are issued back-to-back into
      distinct PSUM banks of one tile, so they run CONCURRENTLY on PE
      row groups 0-63/64-127 (2x) and the slot frees atomically with a
      single FD=1024 exp on ACT (scale=1/8 folded, no max-subtraction).
    - AV(t) is emitted one t late so all 32 matmuls are ready and run
      as dense PE runs (fewer weight-buffer transition stalls).
    - q/k projection mgroups are emitted as half-blocks tucked after the
      AV blocks so the ACT exp chain is never starved at boundaries.
    - normalize: sumexp row DMA-reshaped across 64 DVE lanes for the
      reciprocal, gpsimd partition_broadcast, bf16 throughout.
  phase O : split kk=0..7 partials (bias folded, bf16) that fill the
            ACT-bound t=8/9 stretch, then a short final pass (identity-
            matmul partial re-add + kk=8,9) so the post-attention tail
            is minimal. PE HAM warmup matmuls cover the initial DMA.
"""

import sys

for _p in ("/opt/trn_rl_repo",):
    if _p not in sys.path:
        sys.path.insert(0, _p)

from contextlib import ExitStack

import ml_dtypes
import numpy as np

import concourse.bass as bass  # noqa: F401  (import order: bass before tile)
import concourse.tile as tile
from concourse import bacc, mybir
from concourse.bass_utils import run_bass_kernel_spmd

B, S, D = 8, 1024, 1280
H, HD = 20, 64
SCALING = 0.25  # alpha / rank
ATTN_SCALE = 1.0 / 8.0  # 1/sqrt(HD)

DT = D // 128  # 10 feature tiles
KC = S // 128  # 8 key-position chunks
MG = 5  # weight column groups of 256 (2 output tiles each)
VW = HD + 1  # v columns per head incl. ones column

F32 = mybir.dt.float32
BF16 = mybir.dt.bfloat16
EXP = mybir.ActivationFunctionType.Exp

N_CORES = 8


def _w_stripe(nc, wpool, wdram, mg, name):
    """DMA a 256-wide column group of a weight matrix into SBUF,
    feature-major: stripe[p, t, n] = W[t*128+p, mg*256+n]."""
    stripe = wpool.tile([128, DT, 256], BF16, tag="w", name=name)
    nc.sync.dma_start(
        out=stripe,
        in_=wdram[:, mg * 256 : (mg + 1) * 256].rearrange("(t p) n -> p t n", p=128),
    )
    return stripe


def _qk_mgroup(nc, xT_sb, stripe, pp, dst, mg):
    """One 256-wide column group of a feature-major projection:
    dst[:, m, :] = (W.T @ x.T) for m in the group."""
    for ml in range(2):
        m = mg * 2 + ml
        ps0 = pp.tile([128, 512], F32, tag="pp", name=f"ps0_{m}")
        ps1 = pp.tile([128, 512], F32, tag="pp", name=f"ps1_{m}")
        for kk in range(DT):
            lhsT = stripe[:, kk, ml * 128 : (ml + 1) * 128]
            nc.tensor.matmul(
                ps0, lhsT=lhsT, rhs=xT_sb[:, kk, 0:512],
                start=(kk == 0), stop=(kk == DT - 1),
            )
            nc.tensor.matmul(
                ps1, lhsT=lhsT, rhs=xT_sb[:, kk, 512:1024],
                start=(kk == 0), stop=(kk == DT - 1),
            )
        nc.vector.tensor_copy(out=dst[:, m, 0:512], in_=ps0)
        nc.vector.tensor_copy(out=dst[:, m, 512:1024], in_=ps1)


def _emit(nc, tc, xT, wqT, wkT, wvT, woT, bo, ident, outT):
    persist_cm = tc.tile_pool(name="persist", bufs=1)
    persist = persist_cm.__enter__()
    qT_sb = persist.tile([128, DT, S], BF16)
    kT_sb = persist.tile([128, DT, S], BF16)
    v_sb = persist.tile([128, KC, H * VW], BF16)
    ctxT_sb = persist.tile([128, DT, S], BF16)
    bo_sb = persist.tile([128, DT, 1], F32)
    partial_sb = persist.tile([128, DT, 2, 512], BF16)
    ident_sb = persist.tile([128, 128], BF16)
    nc.sync.dma_start(out=bo_sb, in_=bo[:].rearrange("(t p) -> p t", p=128))
    nc.sync.dma_start(out=ident_sb, in_=ident[:, :])
    nc.vector.memset(
        v_sb[:].rearrange("p a (h c) -> p a h c", c=VW)[:, :, :, HD : HD + 1], 1.0
    )

    xpool_cm = tc.tile_pool(name="xpool", bufs=1)
    xpool = xpool_cm.__enter__()
    xT_sb = xpool.tile([128, DT, S], BF16)

    # wpool opened BEFORE phase V so the first q/k weight stripes do not
    # address-reuse (WAR) the wv region: their DMA can land during phase V.
    wpool_cm = tc.tile_pool(name="wpool", bufs=3)
    wpool = wpool_cm.__enter__()

    # ---------------- phase V: v projection (token-major) ----------------
    with ExitStack() as pv:
        wupool = pv.enter_context(tc.tile_pool(name="wupool", bufs=1))
        wups = pv.enter_context(tc.tile_pool(name="wups", bufs=1, space="PSUM"))
        vwpool = pv.enter_context(tc.tile_pool(name="vwpool", bufs=1))
        vp = pv.enter_context(tc.tile_pool(name="vp", bufs=4, space="PSUM"))
        vw = vwpool.tile([128, DT, D], BF16)

        # HAM warmup: dummy matmuls keep the PE busy during the initial
        # input DMA so real matmuls start at the 2.4 GHz clock.
        wu_sb = wupool.tile([128, 512], BF16)
        nc.vector.memset(wu_sb, 0.0)
        wu_ps = wups.tile([128, 512], F32)
        for _ in range(8):
            nc.tensor.matmul(
                wu_ps, lhsT=wu_sb[:, 0:128], rhs=wu_sb, start=True, stop=True
            )

        # interleave xT / wv tiles so the first V matmuls can start early
        for kk in range(DT):
            nc.sync.dma_start(
                out=xT_sb[:, kk, :], in_=xT[kk * 128 : (kk + 1) * 128, :]
            )
            nc.sync.dma_start(
                out=vw[:, kk, :], in_=wvT[kk * 128 : (kk + 1) * 128, :]
            )
        # prefetch the first q/k stripes behind the V-phase inputs
        sq = _w_stripe(nc, wpool, wqT, 0, "wq0")
        sk = _w_stripe(nc, wpool, wkT, 0, "wk0")
        NCH = ((0, 512), (512, 512), (1024, 256))
        for st in range(KC):
            pss = [
                vp.tile([128, nw], F32, tag="vp", name=f"vps{st}_{ni}")
                for ni, (n0, nw) in enumerate(NCH)
            ]
            for kk in range(DT):
                lhsT = xT_sb[:, kk, st * 128 : (st + 1) * 128]
                for ni, (n0, nw) in enumerate(NCH):
                    nc.tensor.matmul(
                        pss[ni], lhsT=lhsT, rhs=vw[:, kk, n0 : n0 + nw],
                        start=(kk == 0), stop=(kk == DT - 1),
                    )
                if st == 0:
                    # DMA-gated stretch: keep the PE (and HAM) warm with
                    # dependency-free filler matmuls between tile arrivals.
                    for _ in range(4):
                        nc.tensor.matmul(
                            wu_ps, lhsT=wu_sb[:, 0:128], rhs=wu_sb,
                            start=True, stop=True,
                        )
            for ni, (n0, nw) in enumerate(NCH):
                nh = nw // HD
                nc.vector.tensor_copy(
                    out=v_sb[
                        :, st, (n0 // HD) * VW : (n0 // HD + nh) * VW
                    ].rearrange("p (h c) -> p h c", c=VW)[:, :, 0:HD],
                    in_=pss[ni][:].rearrange("p (h c) -> p h c", c=HD),
                )

    # -------- phase QK + attention, interleaved per weight column group ----
    with ExitStack() as p2:
        epool = p2.enter_context(tc.tile_pool(name="epool", bufs=3))
        small = p2.enter_context(tc.tile_pool(name="small", bufs=2))
        pp = p2.enter_context(tc.tile_pool(name="pp", bufs=2, space="PSUM"))
        sc = p2.enter_context(tc.tile_pool(name="sc", bufs=2, space="PSUM"))
        cx = p2.enter_context(tc.tile_pool(name="cx", bufs=2, space="PSUM"))

        def scores_exp(t):
            # exps laid out [128, kc, qc, half, 512] so one FD=1024 exp
            # covers the head pair of each (kc, qc) unit; two tiles per t
            # (kc 0-3 / 4-7) to keep the pool granularity small.
            ex_tiles = [
                epool.tile(
                    [128, KC // 2, 2, 2, 512], BF16, tag="exp", name=f"exp{t}_{i}"
                )
                for i in range(2)
            ]
            # scoresT + exp; head pair (2t, 2t+1) row-packs the PE.
            # The two K=64 halves are issued back-to-back into distinct
            # PSUM banks of ONE tile so they execute concurrently on row
            # groups 0-63 / 64-127 and their slot frees atomically.
            for kc in range(KC):
                for qc in range(2):
                    ps = sc.tile([128, 2, 512], F32, tag="sc", name="scps")
                    for half in range(2):
                        p0 = half * 64
                        nc.tensor.matmul(
                            ps[:, half, :],
                            lhsT=kT_sb[p0 : p0 + 64, t, kc * 128 : (kc + 1) * 128],
                            rhs=qT_sb[p0 : p0 + 64, t, qc * 512 : (qc + 1) * 512],
                            start=True,
                            stop=True,
                        )
                    nc.scalar.activation(
                        out=ex_tiles[kc // 4][:, kc % 4, qc, :, :],
                        in_=ps,
                        func=EXP,
                        scale=ATTN_SCALE,
                    )
            return ex_tiles

        def av_normalize(t, ex_tiles):
            # AV + sumexp + normalize; emitted one t later than its scores
            # so all 32 AV matmuls are ready and run as dense PE runs.
            for half in range(2):
                h = 2 * t + half
                stage = small.tile([HD + 1, S], BF16, tag="stage", name="stage")
                for qc in range(2):
                    cps = cx.tile([HD + 1, 512], F32, tag="cx", name="cxps")
                    for kc in range(KC):
                        nc.tensor.matmul(
                            cps,
                            lhsT=v_sb[:, kc, h * VW : (h + 1) * VW],
                            rhs=ex_tiles[kc // 4][:, kc % 4, qc, half, :],
                            start=(kc == 0),
                            stop=(kc == KC - 1),
                        )
                    nc.vector.tensor_copy(
                        out=stage[:, qc * 512 : (qc + 1) * 512], in_=cps
                    )
                # 1/sumexp: reshape the [1, S] row across 64 DVE lanes
                r64 = small.tile([64, 16], BF16, tag="r64", name="r64")
                nc.sync.dma_start(out=r64, in_=stage[HD : HD + 1, :])
                rc64 = small.tile([64, 16], BF16, tag="rc64", name="rc64")
                with nc.allow_low_precision(reason="bf16 softmax denom"):
                    nc.vector.reciprocal(rc64, r64)
                rrow = small.tile([1, S], BF16, tag="rrow", name="rrow")
                nc.sync.dma_start(out=rrow, in_=rc64)
                bcast = small.tile([HD, S], BF16, tag="bcast", name="bcast")
                nc.gpsimd.partition_broadcast(bcast, rrow)
                nc.vector.tensor_mul(
                    ctxT_sb[half * 64 : half * 64 + 64, t, :],
                    stage[0:HD, :],
                    bcast,
                )

        # mg0 projections run up front; later mgroups are emitted as q/k
        # half-blocks tucked after the AV blocks so the exp (ACT) chain is
        # never starved by a long projection stretch at mgroup boundaries.
        _qk_mgroup(nc, xT_sb, sq, pp, qT_sb, 0)
        _qk_mgroup(nc, xT_sb, sk, pp, kT_sb, 0)
        stripes = {
            (0, 1): _w_stripe(nc, wpool, wqT, 1, "wq1"),
            (1, 1): _w_stripe(nc, wpool, wkT, 1, "wk1"),
        }
        pending = None
        for t in range(DT):
            mgn = t // 2 + 1
            ex = scores_exp(t)
            if pending is not None:
                av_normalize(*pending)
            pending = (t, ex)
            if mgn < MG:
                which = t % 2  # 0 -> q of mgn, 1 -> k of mgn
                dst = qT_sb if which == 0 else kT_sb
                _qk_mgroup(nc, xT_sb, stripes.pop((which, mgn)), pp, dst, mgn)
                if mgn + 1 < MG:
                    wdram, nm = (wqT, "wq") if which == 0 else (wkT, "wk")
                    stripes[(which, mgn + 1)] = _w_stripe(
                        nc, wpool, wdram, mgn + 1, f"{nm}{mgn + 1}"
                    )
        av_normalize(*pending)

        # ---- O partials over kk=0..7 (heads 0..15): pure PE gap-filler for
        # the ACT-bound t=8/9 stretch. Bias is folded into the bf16 partial.
        for mg in range(MG):
            so = _w_stripe(nc, wpool, woT, mg, f"wo{mg}")
            for ml in range(2):
                m = mg * 2 + ml
                for qc in range(2):
                    ps = pp.tile([128, 512], F32, tag="pp", name="opart")
                    for kk in range(DT - 2):
                        nc.tensor.matmul(
                            ps,
                            lhsT=so[:, kk, ml * 128 : (ml + 1) * 128],
                            rhs=ctxT_sb[:, kk, qc * 512 : (qc + 1) * 512],
                            start=(kk == 0),
                            stop=(kk == DT - 3),
                        )
                    nc.vector.tensor_scalar_add(
                        partial_sb[:, m, qc, :], ps, bo_sb[:, m, :]
                    )

    wpool_cm.__exit__(None, None, None)
    xpool_cm.__exit__(None, None, None)

    # ------- phase O final: kk=8,9 + identity-matmul partial add -------
    with ExitStack() as p3:
        wpool3 = p3.enter_context(tc.tile_pool(name="wpool3", bufs=2))
        opp = p3.enter_context(tc.tile_pool(name="opp", bufs=4, space="PSUM"))
        ostage = p3.enter_context(tc.tile_pool(name="ostage", bufs=4))
        for mg in range(MG):
            stripe2 = wpool3.tile([128, 2, 256], BF16, tag="w3", name=f"w3_{mg}")
            nc.sync.dma_start(
                out=stripe2,
                in_=woT[(DT - 2) * 128 :, mg * 256 : (mg + 1) * 256].rearrange(
                    "(t p) n -> p t n", p=128
                ),
            )
            for ml in range(2):
                m = mg * 2 + ml
                for qc in range(2):
                    ps = opp.tile([128, 512], F32, tag="opp", name="opps")
                    nc.tensor.matmul(
                        ps,
                        lhsT=ident_sb,
                        rhs=partial_sb[:, m, qc, :],
                        start=True,
                        stop=False,
                    )
                    for j in range(2):
                        kk = DT - 2 + j
                        nc.tensor.matmul(
                            ps,
                            lhsT=stripe2[:, j, ml * 128 : (ml + 1) * 128],
                            rhs=ctxT_sb[:, kk, qc * 512 : (qc + 1) * 512],
                            start=False,
                            stop=(j == 1),
                        )
                    o_sb = ostage.tile([128, 512], BF16, tag="ostage", name="osb")
                    nc.vector.tensor_copy(o_sb, ps)
                    nc.sync.dma_start(
                        out=outT[m * 128 : (m + 1) * 128, qc * 512 : (qc + 1) * 512],
                        in_=o_sb,
                    )

    persist_cm.__exit__(None, None, None)


def build_nc():
    nc = bacc.Bacc(None, target_bir_lowering=False)
    xT = nc.dram_tensor("xT", [D, S], BF16, kind="ExternalInput")
    wqT = nc.dram_tensor("wqT", [D, D], BF16, kind="ExternalInput")
    wkT = nc.dram_tensor("wkT", [D, D], BF16, kind="ExternalInput")
    wvT = nc.dram_tensor("wvT", [D, D], BF16, kind="ExternalInput")
    woT = nc.dram_tensor("woT", [D, D], BF16, kind="ExternalInput")
    bo = nc.dram_tensor("bo", [D], F32, kind="ExternalInput")
    ident = nc.dram_tensor("ident", [128, 128], BF16, kind="ExternalInput")
    outT = nc.dram_tensor("outT", [D, S], BF16, kind="ExternalOutput")
    with tile.TileContext(nc) as tc:
        _emit(nc, tc, xT, wqT, wkT, wvT, woT, bo, ident, outT)
    nc.compile()
    return nc


_NC = None


def _get_nc():
    global _NC
    if _NC is None:
        _NC = build_nc()
    return _NC


def make_in_maps(hidden_states, Wq, Wk, Wv, Wo, bo, Aq, Bq, Ak, Bk, Av, Bv, Ao, Bo):
    x = np.asarray(hidden_states, dtype=np.float32)

    def eff_T(W, A, Bup):
        W64 = np.asarray(W, dtype=np.float64)
        lora = np.asarray(Bup, dtype=np.float64) @ np.asarray(A, dtype=np.float64)
        return np.ascontiguousarray(
            (W64 + SCALING * lora).T.astype(ml_dtypes.bfloat16)
        )

    base = {
        "wqT": eff_T(Wq, Aq, Bq),
        "wkT": eff_T(Wk, Ak, Bk),
        "wvT": eff_T(Wv, Av, Bv),
        "woT": eff_T(Wo, Ao, Bo),
        "bo": np.ascontiguousarray(np.asarray(bo, dtype=np.float32)),
        "ident": np.eye(128, dtype=ml_dtypes.bfloat16),
    }
    return [
        dict(base, xT=np.ascontiguousarray(x[b].T.astype(ml_dtypes.bfloat16)))
        for b in range(x.shape[0])
    ]


def kernel(**inputs):
    in_maps = make_in_maps(**inputs)
    nc = _get_nc()
    res = run_bass_kernel_spmd(nc, in_maps, core_ids=list(range(N_CORES)))
    out = np.stack([res.results[b]["outT"].T for b in range(N_CORES)])
    return np.ascontiguousarray(out, dtype=np.float32)



# revision 50
# speedup vs baseline: 1.0023x; 1.0004x over previous
"""Trainium2 Bass kernel for a LoRA self-attention block (diffusers-style
CustomLoRAAttnProcessor).

  B=8, S=1024, D=1280, H=20 heads x HD=64, LoRA rank 4 (folded into the
  weights on the host: W_eff = W + 0.25 * B @ A, mathematically identical).

Distribution: pure data parallelism — one batch element per NeuronCore
(8 cores), no collectives.

Per-core layout choices (contraction dim always on SBUF partitions; host
pre-transposes x and the effective weights; all matmul operands bf16 —
fp32 matmuls cost 4x on TRN2):

  phase V : v token-major [S, D] with a ones-column per head (the AV
            matmul then also emits the softmax denominator for free).
  attention pipeline (per feature tile t = head pair 2t, 2t+1):
    - scoresT[k,q]: the two K=64 halves are issued back-to-back into
      distinct PSUM banks of one tile, so they run CONCURRENTLY on PE
      row groups 0-63/64-127 (2x) and the slot frees atomically with a
      single FD=1024 exp on ACT (scale=1/8 folded, no max-subtraction).
    - AV(t) is emitted one t late so all 32 matmuls are ready and run
      as dense PE runs (fewer weight-buffer transition stalls).
    - q/k projection mgroups are emitted as half-blocks tucked after the
      AV blocks so the ACT exp chain is never starved at boundaries.
    - normalize: sumexp row DMA-reshaped across 64 DVE lanes for the
      reciprocal, gpsimd partition_broadcast, bf16 throughout.
  phase O : split kk=0..7 partials (bias folded, bf16) that fill the
            ACT-bound t=8/9 stretch, then a short final pass (identity-
            matmul partial re-add + kk=8,9) so the post-attention tail
            is minimal. PE HAM warmup matmuls cover the initial DMA.
"""

import sys

for _p in ("/opt/trn_rl_repo",):
    if _p not in sys.path:
        sys.path.insert(0, _p)

from contextlib import ExitStack

import ml_dtypes
import numpy as np

import concourse.bass as bass  # noqa: F401  (import order: bass before tile)
import concourse.tile as tile
from concourse import bacc, mybir
from concourse.bass_utils import run_bass_kernel_spmd

B, S, D = 8, 1024, 1280
H, HD = 20, 64
SCALING = 0.25  # alpha / rank
ATTN_SCALE = 1.0 / 8.0  # 1/sqrt(HD)

DT = D // 128  # 10 feature tiles
KC = S // 128  # 8 key-position chunks
MG = 5  # weight column groups of 256 (2 output tiles each)
VW = HD + 1  # v columns per head incl. ones column

F32 = mybir.dt.float32
BF16 = mybir.dt.bfloat16
EXP = mybir.ActivationFunctionType.Exp

N_CORES = 8


def _w_stripe(nc, wpool, wdram, mg, name):
    """DMA a 256-wide column group of a weight matrix into SBUF,
    feature-major: stripe[p, t, n] = W[t*128+p, mg*256+n]."""
    stripe = wpool.tile([128, DT, 256], BF16, tag="w", name=name)
    nc.sync.dma_start(
        out=stripe,
        in_=wdram[:, mg * 256 : (mg + 1) * 256].rearrange("(t p) n -> p t n", p=128),
    )
    return stripe


def _qk_mgroup(nc, xT_sb, stripe, pp, dst, mg):
    """One 256-wide column group of a feature-major projection:
    dst[:, m, :] = (W.T @ x.T) for m in the group."""
    for ml in range(2):
        m = mg * 2 + ml
        ps0 = pp.tile([128, 512], F32, tag="pp", name=f"ps0_{m}")
        ps1 = pp.tile([128, 512], F32, tag="pp", name=f"ps1_{m}")
        for kk in range(DT):
            lhsT = stripe[:, kk, ml * 128 : (ml + 1) * 128]
            nc.tensor.matmul(
                ps0, lhsT=lhsT, rhs=xT_sb[:, kk, 0:512],
                start=(kk == 0), stop=(kk == DT - 1),
            )
            nc.tensor.matmul(
                ps1, lhsT=lhsT, rhs=xT_sb[:, kk, 512:1024],
                start=(kk == 0), stop=(kk == DT - 1),
            )
        nc.vector.tensor_copy(out=dst[:, m, 0:512], in_=ps0)
        nc.vector.tensor_copy(out=dst[:, m, 512:1024], in_=ps1)


def _emit(nc, tc, xT, wqT, wkT, wvT, woT, bo, ident, outT):
    persist_cm = tc.tile_pool(name="persist", bufs=1)
    persist = persist_cm.__enter__()
    qT_sb = persist.tile([128, DT, S], BF16)
    kT_sb = persist.tile([128, DT, S], BF16)
    v_sb = persist.tile([128, KC, H * VW], BF16)
    ctxT_sb = persist.tile([128, DT, S], BF16)
    bo_sb = persist.tile([128, DT, 1], F32)
    partial_sb = persist.tile([128, DT, 2, 512], BF16)
    ident_sb = persist.tile([128, 128], BF16)
    nc.vector.memset(
        v_sb[:].rearrange("p a (h c) -> p a h c", c=VW)[:, :, :, HD : HD + 1], 1.0
    )

    xpool_cm = tc.tile_pool(name="xpool", bufs=1)
    xpool = xpool_cm.__enter__()
    xT_sb = xpool.tile([128, DT, S], BF16)

    # wpool opened BEFORE phase V so the first q/k weight stripes do not
    # address-reuse (WAR) the wv region: their DMA can land during phase V.
    wpool_cm = tc.tile_pool(name="wpool", bufs=3)
    wpool = wpool_cm.__enter__()

    # ---------------- phase V: v projection (token-major) ----------------
    with ExitStack() as pv:
        wupool = pv.enter_context(tc.tile_pool(name="wupool", bufs=1))
        wups = pv.enter_context(tc.tile_pool(name="wups", bufs=1, space="PSUM"))
        vwpool = pv.enter_context(tc.tile_pool(name="vwpool", bufs=1))
        vp = pv.enter_context(tc.tile_pool(name="vp", bufs=4, space="PSUM"))
        vw = vwpool.tile([128, DT, D], BF16)

        # HAM warmup: dummy matmuls keep the PE busy during the initial
        # input DMA so real matmuls start at the 2.4 GHz clock.
        wu_sb = wupool.tile([128, 512], BF16)
        nc.vector.memset(wu_sb, 0.0)
        wu_ps = wups.tile([128, 512], F32)
        for _ in range(8):
            nc.tensor.matmul(
                wu_ps, lhsT=wu_sb[:, 0:128], rhs=wu_sb, start=True, stop=True
            )

        # interleave xT / wv tiles so the first V matmuls can start early
        for kk in range(DT):
            nc.sync.dma_start(
                out=xT_sb[:, kk, :], in_=xT[kk * 128 : (kk + 1) * 128, :]
            )
            nc.sync.dma_start(
                out=vw[:, kk, :], in_=wvT[kk * 128 : (kk + 1) * 128, :]
            )
        # prefetch the first q/k stripes behind the V-phase inputs;
        # bo/ident are not needed until the O phase, so they queue last
        sq = _w_stripe(nc, wpool, wqT, 0, "wq0")
        sk = _w_stripe(nc, wpool, wkT, 0, "wk0")
        nc.sync.dma_start(out=bo_sb, in_=bo[:].rearrange("(t p) -> p t", p=128))
        nc.sync.dma_start(out=ident_sb, in_=ident[:, :])
        NCH = ((0, 512), (512, 512), (1024, 256))
        for st in range(KC):
            pss = [
                vp.tile([128, nw], F32, tag="vp", name=f"vps{st}_{ni}")
                for ni, (n0, nw) in enumerate(NCH)
            ]
            for kk in range(DT):
                lhsT = xT_sb[:, kk, st * 128 : (st + 1) * 128]
                for ni, (n0, nw) in enumerate(NCH):
                    nc.tensor.matmul(
                        pss[ni], lhsT=lhsT, rhs=vw[:, kk, n0 : n0 + nw],
                        start=(kk == 0), stop=(kk == DT - 1),
                    )
                if st == 0:
                    # DMA-gated stretch: keep the PE (and HAM) warm with
                    # dependency-free filler matmuls between tile arrivals.
                    for _ in range(4):
                        nc.tensor.matmul(
                            wu_ps, lhsT=wu_sb[:, 0:128], rhs=wu_sb,
                            start=True, stop=True,
                        )
            for ni, (n0, nw) in enumerate(NCH):
                nh = nw // HD
                nc.vector.tensor_copy(
                    out=v_sb[
                        :, st, (n0 // HD) * VW : (n0 // HD + nh) * VW
                    ].rearrange("p (h c) -> p h c", c=VW)[:, :, 0:HD],
                    in_=pss[ni][:].rearrange("p (h c) -> p h c", c=HD),
                )

    # -------- phase QK + attention, interleaved per weight column group ----
    with ExitStack() as p2:
        epool = p2.enter_context(tc.tile_pool(name="epool", bufs=3))
        small = p2.enter_context(tc.tile_pool(name="small", bufs=2))
        pp = p2.enter_context(tc.tile_pool(name="pp", bufs=2, space="PSUM"))
        sc = p2.enter_context(tc.tile_pool(name="sc", bufs=2, space="PSUM"))
        cx = p2.enter_context(tc.tile_pool(name="cx", bufs=2, space="PSUM"))

        def scores_exp(t):
            # exps laid out [128, kc, qc, half, 512] so one FD=1024 exp
            # covers the head pair of each (kc, qc) unit; two tiles per t
            # (kc 0-3 / 4-7) to keep the pool granularity small.
            ex_tiles = [
                epool.tile(
                    [128, KC // 2, 2, 2, 512], BF16, tag="exp", name=f"exp{t}_{i}"
                )
                for i in range(2)
            ]
            # scoresT + exp; head pair (2t, 2t+1) row-packs the PE.
            # The two K=64 halves are issued back-to-back into distinct
            # PSUM banks of ONE tile so they execute concurrently on row
            # groups 0-63 / 64-127 and their slot frees atomically.
            for kc in range(KC):
                for qc in range(2):
                    ps = sc.tile([128, 2, 512], F32, tag="sc", name="scps")
                    for half in range(2):
                        p0 = half * 64
                        nc.tensor.matmul(
                            ps[:, half, :],
                            lhsT=kT_sb[p0 : p0 + 64, t, kc * 128 : (kc + 1) * 128],
                            rhs=qT_sb[p0 : p0 + 64, t, qc * 512 : (qc + 1) * 512],
                            start=True,
                            stop=True,
                        )
                    nc.scalar.activation(
                        out=ex_tiles[kc // 4][:, kc % 4, qc, :, :],
                        in_=ps,
                        func=EXP,
                        scale=ATTN_SCALE,
                    )
            return ex_tiles

        def av_normalize(t, ex_tiles):
            # AV + sumexp + normalize; emitted one t later than its scores
            # so all 32 AV matmuls are ready and run as dense PE runs.
            for half in range(2):
                h = 2 * t + half
                stage = small.tile([HD + 1, S], BF16, tag="stage", name="stage")
                for qc in range(2):
                    cps = cx.tile([HD + 1, 512], F32, tag="cx", name="cxps")
                    for kc in range(KC):
                        nc.tensor.matmul(
                            cps,
                            lhsT=v_sb[:, kc, h * VW : (h + 1) * VW],
                            rhs=ex_tiles[kc // 4][:, kc % 4, qc, half, :],
                            start=(kc == 0),
                            stop=(kc == KC - 1),
                        )
                    nc.vector.tensor_copy(
                        out=stage[:, qc * 512 : (qc + 1) * 512], in_=cps
                    )
                # 1/sumexp: reshape the [1, S] row across 64 DVE lanes
                r64 = small.tile([64, 16], BF16, tag="r64", name="r64")
                nc.sync.dma_start(out=r64, in_=stage[HD : HD + 1, :])
                rc64 = small.tile([64, 16], BF16, tag="rc64", name="rc64")
                with nc.allow_low_precision(reason="bf16 softmax denom"):
                    nc.vector.reciprocal(rc64, r64)
                rrow = small.tile([1, S], BF16, tag="rrow", name="rrow")
                nc.sync.dma_start(out=rrow, in_=rc64)
                bcast = small.tile([HD, S], BF16, tag="bcast", name="bcast")
                nc.gpsimd.partition_broadcast(bcast, rrow)
                nc.vector.tensor_mul(
                    ctxT_sb[half * 64 : half * 64 + 64, t, :],
                    stage[0:HD, :],
                    bcast,
                )

        # mg0 projections run up front; later mgroups are emitted as q/k
        # half-blocks tucked after the AV blocks so the exp (ACT) chain is
        # never starved by a long projection stretch at mgroup boundaries.
        _qk_mgroup(nc, xT_sb, sq, pp, qT_sb, 0)
        _qk_mgroup(nc, xT_sb, sk, pp, kT_sb, 0)
        stripes = {
            (0, 1): _w_stripe(nc, wpool, wqT, 1, "wq1"),
            (1, 1): _w_stripe(nc, wpool, wkT, 1, "wk1"),
        }
        pending = None
        for t in range(DT):
            mgn = t // 2 + 1
            ex = scores_exp(t)
            if pending is not None:
                av_normalize(*pending)
            pending = (t, ex)
            if mgn < MG:
                which = t % 2  # 0 -> q of mgn, 1 -> k of mgn
                dst = qT_sb if which == 0 else kT_sb
                _qk_mgroup(nc, xT_sb, stripes.pop((which, mgn)), pp, dst, mgn)
                if mgn + 1 < MG:
                    wdram, nm = (wqT, "wq") if which == 0 else (wkT, "wk")
                    stripes[(which, mgn + 1)] = _w_stripe(
                        nc, wpool, wdram, mgn + 1, f"{nm}{mgn + 1}"
                    )
        av_normalize(*pending)

        # ---- O partials over kk=0..7 (heads 0..15): pure PE gap-filler for
        # the ACT-bound t=8/9 stretch. Bias is folded into the bf16 partial.
        for mg in range(MG):
            so = _w_stripe(nc, wpool, woT, mg, f"wo{mg}")
            for ml in range(2):
                m = mg * 2 + ml
                for qc in range(2):
                    ps = pp.tile([128, 512], F32, tag="pp", name="opart")
                    for kk in range(DT - 2):
                        nc.tensor.matmul(
                            ps,
                            lhsT=so[:, kk, ml * 128 : (ml + 1) * 128],
                            rhs=ctxT_sb[:, kk, qc * 512 : (qc + 1) * 512],
                            start=(kk == 0),
                            stop=(kk == DT - 3),
                        )
                    nc.vector.tensor_scalar_add(
                        partial_sb[:, m, qc, :], ps, bo_sb[:, m, :]
                    )

    wpool_cm.__exit__(None, None, None)
    xpool_cm.__exit__(None, None, None)

    # ------- phase O final: kk=8,9 + identity-matmul partial add -------
    with ExitStack() as p3:
        wpool3 = p3.enter_context(tc.tile_pool(name="wpool3", bufs=2))
        opp = p3.enter_context(tc.tile_pool(name="opp", bufs=4, space="PSUM"))
        ostage = p3.enter_context(tc.tile_pool(name="ostage", bufs=4))
        for mg in range(MG):
            stripe2 = wpool3.tile([128, 2, 256], BF16, tag="w3", name=f"w3_{mg}")
            nc.sync.dma_start(
                out=stripe2,
                in_=woT[(DT - 2) * 128 :, mg * 256 : (mg + 1) * 256].rearrange(
                    "(t p) n -> p t n", p=128
                ),
            )
            for ml in range(2):
                m = mg * 2 + ml
                for qc in range(2):
                    ps = opp.tile([128, 512], F32, tag="opp", name="opps")
                    nc.tensor.matmul(
                        ps,
                        lhsT=ident_sb,
                        rhs=partial_sb[:, m, qc, :],
                        start=True,
                        stop=False,
                    )
                    for j in range(2):
                        kk = DT - 2 + j
                        nc.tensor.matmul(
                            ps,
                            lhsT=stripe2[:, j, ml * 128 : (ml + 1) * 128],
                            rhs=ctxT_sb[:, kk, qc * 512 : (qc + 1) * 512],
                            start=False,
                            stop=(j == 1),
                        )
                    o_sb = ostage.tile([128, 512], BF16, tag="ostage", name="osb")
                    nc.vector.tensor_copy(o_sb, ps)
                    nc.sync.dma_start(
                        out=outT[m * 128 : (m + 1) * 128, qc * 512 : (qc + 1) * 512],
                        in_=o_sb,
                    )

    persist_cm.__exit__(None, None, None)


def build_nc():
    nc = bacc.Bacc(None, target_bir_lowering=False)
    xT = nc.dram_tensor("xT", [D, S], BF16, kind="ExternalInput")
    wqT = nc.dram_tensor("wqT", [D, D], BF16, kind="ExternalInput")
    wkT = nc.dram_tensor("wkT", [D, D], BF16, kind="ExternalInput")
    wvT = nc.dram_tensor("wvT", [D, D], BF16, kind="ExternalInput")
    woT = nc.dram_tensor("woT", [D, D], BF16, kind="ExternalInput")
    bo = nc.dram_tensor("bo", [D], F32, kind="ExternalInput")
    ident = nc.dram_tensor("ident", [128, 128], BF16, kind="ExternalInput")
    outT = nc.dram_tensor("outT", [D, S], BF16, kind="ExternalOutput")
    with tile.TileContext(nc) as tc:
        _emit(nc, tc, xT, wqT, wkT, wvT, woT, bo, ident, outT)
    nc.compile()
    return nc


_NC = None


def _get_nc():
    global _NC
    if _NC is None:
        _NC = build_nc()
    return _NC


def make_in_maps(hidden_states, Wq, Wk, Wv, Wo, bo, Aq, Bq, Ak, Bk, Av, Bv, Ao, Bo):
    x = np.asarray(hidden_states, dtype=np.float32)

    def eff_T(W, A, Bup):
        W64 = np.asarray(W, dtype=np.float64)
        lora = np.asarray(Bup, dtype=np.float64) @ np.asarray(A, dtype=np.float64)
        return np.ascontiguousarray(
            (W64 + SCALING * lora).T.astype(ml_dtypes.bfloat16)
        )

    base = {
        "wqT": eff_T(Wq, Aq, Bq),
        "wkT": eff_T(Wk, Ak, Bk),
        "wvT": eff_T(Wv, Av, Bv),
        "woT": eff_T(Wo, Ao, Bo),
        "bo": np.ascontiguousarray(np.asarray(bo, dtype=np.float32)),
        "ident": np.eye(128, dtype=ml_dtypes.bfloat16),
    }
    return [
        dict(base, xT=np.ascontiguousarray(x[b].T.astype(ml_dtypes.bfloat16)))
        for b in range(x.shape[0])
    ]


def kernel(**inputs):
    in_maps = make_in_maps(**inputs)
    nc = _get_nc()
    res = run_bass_kernel_spmd(nc, in_maps, core_ids=list(range(N_CORES)))
    out = np.stack([res.results[b]["outT"].T for b in range(N_CORES)])
    return np.ascontiguousarray(out, dtype=np.float32)

